# revision 11
# baseline (speedup 1.0000x reference)
"""GRU cell kernel for Trainium2, data-parallel over 8 NeuronCores.

Math (per batch row):
    x_proj = x @ W_ih.T + b           -> r_x, z_x, n_x
    r = sigmoid(r_x + h @ U_r.T)
    z = sigmoid(z_x + h @ U_z.T)
    n = tanh(n_x + r * (h @ U_n.T + U_n_b))
    out = (1 - z) * n + z * h

Layout strategy: all on-chip compute happens in "transposed" orientation so
both matmul operands carry the contraction dim H on the partition axis:
  - host sends x.T, h.T slices per core ([H, B_local]) and pre-packed
    transposed weights; kernel computes out.T tiles [o_feat=128, batch=512]
  - bf16 matmuls (full PE rate), fp32 PSUM accumulation, fp32 epilogue
  - host transposes the per-core [H, B_local] outputs back at the end
"""

import os
import numpy as np
import ml_dtypes

import concourse.bass as bass
import concourse.mybir as mybir
import concourse.tile as tile
from concourse import bacc
from concourse.bass_utils import run_bass_kernel_spmd

H = 1024
B = 8192
NCORES = 8
BL = B // NCORES          # batch rows per core
KT = H // 128             # contraction k-tiles
OT = H // 128             # output-feature tiles (per gate)
NB = BL // 512            # batch slices of 512
F32 = mybir.dt.float32
BF16 = mybir.dt.bfloat16
BF16_NP = ml_dtypes.bfloat16

# gate order inside the packed weight tensor's 768-wide free dim
# g: 0=W_r 1=W_z 2=W_n 3=U_r 4=U_z 5=U_n

LAST_RESULT = None  # BassKernelResults of the most recent run (for test harness)


def _gru_tile_kernel(tc, outt, xh, htf, wp, bias_ap):
    nc = tc.nc
    sig = mybir.ActivationFunctionType.Sigmoid
    tanh = mybir.ActivationFunctionType.Tanh
    add = mybir.AluOpType.add
    mult = mybir.AluOpType.mult

    from contextlib import ExitStack

    with ExitStack() as ctx:
        singles = ctx.enter_context(tc.tile_pool(name="singles", bufs=1))
        wpool = ctx.enter_context(tc.tile_pool(name="wpool", bufs=2))
        gates = ctx.enter_context(tc.tile_pool(name="gates", bufs=2))
        outp = ctx.enter_context(tc.tile_pool(name="outp", bufs=3))
        psum = ctx.enter_context(tc.tile_pool(name="psum", bufs=2, space="PSUM"))

        # resident activations: x.T and h.T bf16 packed per k-tile as
        # [x_b0 | h_b0 | x_b1 | h_b1] so one DMA covers a whole batch-half
        xh_t = [
            singles.tile([128, 2 * BL], BF16, name=f"xh{k}", tag=f"xh{k}")
            for k in range(KT)
        ]
        hf_t = [singles.tile([128, BL], F32, name=f"hf{k}", tag=f"hf{k}") for k in range(KT)]
        bias_t = singles.tile([128, OT * 4], F32, name="bias", tag="bias")
        warm_sb = singles.tile([128, 512], BF16, name="warm_sb", tag="warm_sb")

        xh3 = xh.rearrange("(kt p) b -> kt p b", p=128)
        hf3 = htf.rearrange("(kt p) b -> kt p b", p=128)

        # DMA-trigger issue costs ~0.6us per dma_start on an engine queue,
        # so triggers are spread over three independent paths and ordered
        # first-needed-first:
        #   scalar ring : packed weights (per-k chunks for o=0 only)
        #   sync ring   : packed x/h bf16 halves (b0 first), then fp32 h.T
        #   gpsimd SWDGE: output stores
        nc.gpsimd.dma_start(out=bias_t[:], in_=bias_ap[:])
        for half in range(NB):
            cs = bass.ts(half, 1024)
            for k in range(KT):
                nc.sync.dma_start(out=xh_t[k][:, cs], in_=xh3[k][:, cs])
        for k in range(KT):
            nc.sync.dma_start(out=hf_t[k][:], in_=hf3[k])

        # warm the PE clock (HAM) with dummy matmuls on a memset tile so
        # the real matmul stream starts at full clock
        nc.gpsimd.memset(warm_sb[:], 0.0)
        warm_ps = psum.tile([128, 512], F32, name="warm_ps", tag="r_ps")
        for _ in range(10):
            nc.tensor.matmul(
                warm_ps[:], warm_sb[:, 0:128], warm_sb[:], start=True, stop=True
            )

        for o in range(OT):
            # packed weights for this output-feature tile: [128, kt, 6*128].
            # o=0 is latency-critical: per-k 192KB contiguous chunks so
            # matmul k waits only on chunk k. Later o: two triggers each.
            wt = wpool.tile([128, KT, 6 * 128], BF16, name="wt", tag="wt")
            if o == 0:
                wp_o = wp[o].rearrange("(kt p) f -> kt p f", p=128)
                for k in range(KT):
                    nc.scalar.dma_start(out=wt[:, k, :], in_=wp_o[k])
            else:
                wp_o = wp[o].rearrange("(h kt p) f -> h p kt f", h=2, p=128)
                nc.scalar.dma_start(out=wt[:, 0:4, :], in_=wp_o[0])
                nc.scalar.dma_start(out=wt[:, 4:8, :], in_=wp_o[1])

            for b in range(NB):
                xs = bass.ds(b * 1024, 512)        # x columns of this half
                hs = bass.ds(b * 1024 + 512, 512)  # h columns of this half
                bs = bass.ts(b, 512)
                r_ps = psum.tile([128, 512], F32, name="r_ps", tag="r_ps")
                z_ps = psum.tile([128, 512], F32, name="z_ps", tag="z_ps")
                nx_ps = psum.tile([128, 512], F32, name="nx_ps", tag="nx_ps")
                nh_ps = psum.tile([128, 512], F32, name="nh_ps", tag="nh_ps")

                def mm_group(ps, parts):
                    n_mm = KT * len(parts)
                    i = 0
                    for g, cols in parts:
                        for k in range(KT):
                            nc.tensor.matmul(
                                ps[:],
                                wt[:, k, g * 128 : (g + 1) * 128],
                                xh_t[k][:, cols],
                                start=(i == 0),
                                stop=(i == n_mm - 1),
                            )
                            i += 1

                # group order nx, nh, r, z: the n/tanh chain then overlaps
                # the z matmuls, shrinking the per-unit (and kernel) tail
                mm_group(nx_ps, [(2, xs)])
                mm_group(nh_ps, [(5, hs)])
                mm_group(r_ps, [(0, xs), (3, hs)])

                r_sb = gates.tile([128, 512], F32, name="r", tag="r")
                nc.scalar.activation(
                    out=r_sb[:], in_=r_ps[:], func=sig,
                    bias=bias_t[:, o * 4 + 0 : o * 4 + 1],
                )

                mm_group(z_ps, [(1, xs), (4, hs)])

                # t = (n_h + b_n2) * r ; s = n_x + t ; n = tanh(s + b_n1)
                # d = h - n    (all run while the z matmuls stream)
                t_sb = gates.tile([128, 512], F32, name="t", tag="t")
                nc.vector.scalar_tensor_tensor(
                    out=t_sb[:], in0=nh_ps[:],
                    scalar=bias_t[:, o * 4 + 3 : o * 4 + 4],
                    in1=r_sb[:], op0=add, op1=mult,
                )
                s_sb = gates.tile([128, 512], F32, name="s", tag="s")
                nc.vector.tensor_add(s_sb[:], nx_ps[:], t_sb[:])
                n_sb = gates.tile([128, 512], F32, name="n", tag="n")
                nc.scalar.activation(
                    out=n_sb[:], in_=s_sb[:], func=tanh,
                    bias=bias_t[:, o * 4 + 2 : o * 4 + 3],
                )
                d_sb = gates.tile([128, 512], F32, name="d", tag="d")
                nc.vector.tensor_sub(d_sb[:], hf_t[o][:, bs], n_sb[:])

                # post-z chain in two column chunks so the final chunk's
                # serial latency (and the kernel tail) is halved:
                # z = sigmoid(z_pre + b_z); out = n + z * d
                z_sb = gates.tile([128, 512], F32, name="z", tag="z")
                p_sb = gates.tile([128, 512], F32, name="p", tag="p")
                o_sb = outp.tile([128, 512], F32, name="o", tag="o")
                for c in range(2):
                    cc = bass.ts(c, 256)
                    nc.scalar.activation(
                        out=z_sb[:, cc], in_=z_ps[:, cc], func=sig,
                        bias=bias_t[:, o * 4 + 1 : o * 4 + 2],
                    )
                    nc.vector.tensor_mul(p_sb[:, cc], z_sb[:, cc], d_sb[:, cc])
                    nc.vector.tensor_add(o_sb[:, cc], n_sb[:, cc], p_sb[:, cc])
                    # late stores ride the (idle by then) sync HWDGE ring,
                    # whose completion receipt is faster than SWDGE
                    store_eng = nc.sync if o >= 4 else nc.gpsimd
                    store_eng.dma_start(
                        out=outt[
                            o * 128 : (o + 1) * 128,
                            b * 512 + c * 256 : b * 512 + (c + 1) * 256,
                        ],
                        in_=o_sb[:, cc],
                    )


_NC_CACHE = None


def _build_nc():
    global _NC_CACHE
    if _NC_CACHE is not None:
        return _NC_CACHE
    nc = bacc.Bacc(
        "TRN2", target_bir_lowering=False, debug=False, num_devices=NCORES
    )
    xh = nc.dram_tensor("xh", [H, 2 * BL], BF16, kind="ExternalInput").ap()
    htf = nc.dram_tensor("htf", [H, BL], F32, kind="ExternalInput").ap()
    wp = nc.dram_tensor("wp", [OT, H, 6 * 128], BF16, kind="ExternalInput").ap()
    bias = nc.dram_tensor("bias", [128, OT * 4], F32, kind="ExternalInput").ap()
    outt = nc.dram_tensor("outt", [H, BL], F32, kind="ExternalOutput").ap()

    with tile.TileContext(nc) as tc:
        _gru_tile_kernel(tc, outt, xh, htf, wp, bias)
    nc.compile()
    _NC_CACHE = nc
    return nc


def _pack_inputs(x, h, W_ih_w, W_ih_b, U_r_w, U_z_w, U_n_w, U_n_b):
    x = np.asarray(x, dtype=np.float32)
    h = np.asarray(h, dtype=np.float32)
    xT = np.ascontiguousarray(x.T)                      # [H, B]
    hT = np.ascontiguousarray(h.T)
    xTb = xT.astype(BF16_NP)
    hTb = hT.astype(BF16_NP)

    W_all = np.concatenate(
        [np.asarray(W_ih_w, np.float32)] +
        [np.asarray(u, np.float32) for u in (U_r_w, U_z_w, U_n_w)],
        axis=0,
    )                                                   # [6H, H] rows: Wr Wz Wn Ur Uz Un
    WT = np.ascontiguousarray(W_all.T)                  # [H, 6H], col blocks same order
    # wp[o, k, g*128 + m] = WT[k, g*H + o*128 + m]
    wp = np.ascontiguousarray(
        WT.reshape(H, 6, OT, 128).transpose(2, 0, 1, 3).reshape(OT, H, 6 * 128)
    ).astype(BF16_NP)

    b_all = np.concatenate(
        [np.asarray(W_ih_b, np.float32), np.asarray(U_n_b, np.float32)]
    )                                                   # [4H]: b_r b_z b_n1 b_n2
    # bias[m, o*4 + g] = b_all[g*H + o*128 + m]
    bias = np.ascontiguousarray(
        b_all.reshape(4, OT, 128).transpose(2, 1, 0).reshape(128, OT * 4)
    ).astype(np.float32)

    in_maps = []
    for c in range(NCORES):
        sl = slice(c * BL, (c + 1) * BL)
        xc, hc = xTb[:, sl], hTb[:, sl]
        # per-k rows packed as [x_b0 | h_b0 | x_b1 | h_b1]
        xhc = np.concatenate(
            [xc[:, 0:512], hc[:, 0:512], xc[:, 512:1024], hc[:, 512:1024]],
            axis=1,
        )
        in_maps.append({
            "xh": np.ascontiguousarray(xhc),
            "htf": np.ascontiguousarray(hT[:, sl]),
            "wp": wp,
            "bias": bias,
        })
    return in_maps


def kernel(x, h, W_ih_w, W_ih_b, U_r_w, U_z_w, U_n_w, U_n_b):
    global LAST_RESULT
    nc = _build_nc()
    in_maps = _pack_inputs(x, h, W_ih_w, W_ih_b, U_r_w, U_z_w, U_n_w, U_n_b)
    trace = bool(os.environ.get("GRU_TRACE"))
    res = run_bass_kernel_spmd(nc, in_maps, list(range(NCORES)), trace=trace)
    LAST_RESULT = res
    out = np.empty((B, H), dtype=np.float32)
    for c in range(NCORES):
        out[c * BL : (c + 1) * BL, :] = res.results[c]["outt"].T
    return out


# revision 12
# speedup vs baseline: 1.0080x; 1.0080x over previous
"""GRU cell kernel for Trainium2, data-parallel over 8 NeuronCores.

Math (per batch row):
    x_proj = x @ W_ih.T + b           -> r_x, z_x, n_x
    r = sigmoid(r_x + h @ U_r.T)
    z = sigmoid(z_x + h @ U_z.T)
    n = tanh(n_x + r * (h @ U_n.T + U_n_b))
    out = (1 - z) * n + z * h

Layout strategy: all on-chip compute happens in "transposed" orientation so
both matmul operands carry the contraction dim H on the partition axis:
  - host sends x.T, h.T slices per core ([H, B_local]) and pre-packed
    transposed weights; kernel computes out.T tiles [o_feat=128, batch=512]
  - bf16 matmuls (full PE rate), fp32 PSUM accumulation, fp32 epilogue
  - host transposes the per-core [H, B_local] outputs back at the end
"""

import os
import numpy as np
import ml_dtypes

import concourse.bass as bass
import concourse.mybir as mybir
import concourse.tile as tile
from concourse import bacc
from concourse.bass_utils import run_bass_kernel_spmd

H = 1024
B = 8192
NCORES = 8
BL = B // NCORES          # batch rows per core
KT = H // 128             # contraction k-tiles
OT = H // 128             # output-feature tiles (per gate)
NB = BL // 512            # batch slices of 512
F32 = mybir.dt.float32
BF16 = mybir.dt.bfloat16
BF16_NP = ml_dtypes.bfloat16

# gate order inside the packed weight tensor's 768-wide free dim
# g: 0=W_r 1=W_z 2=W_n 3=U_r 4=U_z 5=U_n

LAST_RESULT = None  # BassKernelResults of the most recent run (for test harness)


def _gru_tile_kernel(tc, outt, xh, htf, wp, bias_ap):
    nc = tc.nc
    sig = mybir.ActivationFunctionType.Sigmoid
    tanh = mybir.ActivationFunctionType.Tanh
    add = mybir.AluOpType.add
    mult = mybir.AluOpType.mult

    from contextlib import ExitStack

    with ExitStack() as ctx:
        singles = ctx.enter_context(tc.tile_pool(name="singles", bufs=1))
        wpool = ctx.enter_context(tc.tile_pool(name="wpool", bufs=2))
        gates = ctx.enter_context(tc.tile_pool(name="gates", bufs=2))
        outp = ctx.enter_context(tc.tile_pool(name="outp", bufs=3))
        psum = ctx.enter_context(tc.tile_pool(name="psum", bufs=2, space="PSUM"))

        # resident activations: x.T and h.T bf16 packed per k-tile as
        # [x_b0 | h_b0 | x_b1 | h_b1] so one DMA covers a whole batch-half
        xh_t = [
            singles.tile([128, 2 * BL], BF16, name=f"xh{k}", tag=f"xh{k}")
            for k in range(KT)
        ]
        hf_t = [singles.tile([128, BL], F32, name=f"hf{k}", tag=f"hf{k}") for k in range(KT)]
        bias_t = singles.tile([128, OT * 4], F32, name="bias", tag="bias")
        warm_sb = singles.tile([128, 512], BF16, name="warm_sb", tag="warm_sb")

        xh3 = xh.rearrange("(kt p) b -> kt p b", p=128)
        hf3 = htf.rearrange("(kt p) b -> kt p b", p=128)

        # DMA-trigger issue costs ~0.6us per dma_start on an engine queue,
        # so triggers are spread over three independent paths and ordered
        # first-needed-first:
        #   scalar ring : packed weights (per-k chunks for o=0 only)
        #   sync ring   : packed x/h bf16 halves (b0 first), then fp32 h.T
        #   gpsimd SWDGE: output stores
        # warm the PE clock (HAM) with dummy matmuls on a memset tile so
        # the real matmul stream starts at full clock
        nc.gpsimd.memset(warm_sb[:], 0.0)
        warm_ps = psum.tile([128, 512], F32, name="warm_ps", tag="r_ps")
        for _ in range(10):
            nc.tensor.matmul(
                warm_ps[:], warm_sb[:, 0:128], warm_sb[:], start=True, stop=True
            )

        nc.gpsimd.dma_start(out=bias_t[:], in_=bias_ap[:])
        for half in range(NB):
            cs = bass.ts(half, 1024)
            for k in range(KT):
                nc.sync.dma_start(out=xh_t[k][:, cs], in_=xh3[k][:, cs])
        for k in range(KT):
            nc.sync.dma_start(out=hf_t[k][:], in_=hf3[k])

        for o in range(OT):
            # packed weights for this output-feature tile: [128, kt, 6*128].
            # o=0 is latency-critical: per-k 192KB contiguous chunks so
            # matmul k waits only on chunk k. Later o: two triggers each.
            wt = wpool.tile([128, KT, 6 * 128], BF16, name="wt", tag="wt")
            if o <= 1:
                wp_o = wp[o].rearrange("(kt p) f -> kt p f", p=128)
                for k in range(KT):
                    nc.scalar.dma_start(out=wt[:, k, :], in_=wp_o[k])
            else:
                wp_o = wp[o].rearrange("(h kt p) f -> h p kt f", h=2, p=128)
                nc.scalar.dma_start(out=wt[:, 0:4, :], in_=wp_o[0])
                nc.scalar.dma_start(out=wt[:, 4:8, :], in_=wp_o[1])

            for b in range(NB):
                xs = bass.ds(b * 1024, 512)        # x columns of this half
                hs = bass.ds(b * 1024 + 512, 512)  # h columns of this half
                bs = bass.ts(b, 512)
                r_ps = psum.tile([128, 512], F32, name="r_ps", tag="r_ps")
                z_ps = psum.tile([128, 512], F32, name="z_ps", tag="z_ps")
                nx_ps = psum.tile([128, 512], F32, name="nx_ps", tag="nx_ps")
                nh_ps = psum.tile([128, 512], F32, name="nh_ps", tag="nh_ps")

                def mm_group(ps, parts):
                    n_mm = KT * len(parts)
                    i = 0
                    for g, cols in parts:
                        for k in range(KT):
                            nc.tensor.matmul(
                                ps[:],
                                wt[:, k, g * 128 : (g + 1) * 128],
                                xh_t[k][:, cols],
                                start=(i == 0),
                                stop=(i == n_mm - 1),
                            )
                            i += 1

                # group order nx, nh, r, z: the n/tanh chain then overlaps
                # the z matmuls, shrinking the per-unit (and kernel) tail
                mm_group(nx_ps, [(2, xs)])
                mm_group(nh_ps, [(5, hs)])
                mm_group(r_ps, [(0, xs), (3, hs)])

                r_sb = gates.tile([128, 512], F32, name="r", tag="r")
                nc.scalar.activation(
                    out=r_sb[:], in_=r_ps[:], func=sig,
                    bias=bias_t[:, o * 4 + 0 : o * 4 + 1],
                )

                mm_group(z_ps, [(1, xs), (4, hs)])

                # t = (n_h + b_n2) * r ; s = n_x + t ; n = tanh(s + b_n1)
                # d = h - n    (all run while the z matmuls stream)
                t_sb = gates.tile([128, 512], F32, name="t", tag="t")
                nc.vector.scalar_tensor_tensor(
                    out=t_sb[:], in0=nh_ps[:],
                    scalar=bias_t[:, o * 4 + 3 : o * 4 + 4],
                    in1=r_sb[:], op0=add, op1=mult,
                )
                s_sb = gates.tile([128, 512], F32, name="s", tag="s")
                nc.vector.tensor_add(s_sb[:], nx_ps[:], t_sb[:])
                n_sb = gates.tile([128, 512], F32, name="n", tag="n")
                nc.scalar.activation(
                    out=n_sb[:], in_=s_sb[:], func=tanh,
                    bias=bias_t[:, o * 4 + 2 : o * 4 + 3],
                )
                d_sb = gates.tile([128, 512], F32, name="d", tag="d")
                nc.vector.tensor_sub(d_sb[:], hf_t[o][:, bs], n_sb[:])

                # post-z chain in two column chunks so the final chunk's
                # serial latency (and the kernel tail) is halved:
                # z = sigmoid(z_pre + b_z); out = n + z * d
                z_sb = gates.tile([128, 512], F32, name="z", tag="z")
                p_sb = gates.tile([128, 512], F32, name="p", tag="p")
                o_sb = outp.tile([128, 512], F32, name="o", tag="o")
                for c in range(2):
                    cc = bass.ts(c, 256)
                    nc.scalar.activation(
                        out=z_sb[:, cc], in_=z_ps[:, cc], func=sig,
                        bias=bias_t[:, o * 4 + 1 : o * 4 + 2],
                    )
                    nc.vector.tensor_mul(p_sb[:, cc], z_sb[:, cc], d_sb[:, cc])
                    nc.vector.tensor_add(o_sb[:, cc], n_sb[:, cc], p_sb[:, cc])
                    # late stores ride the (idle by then) sync HWDGE ring,
                    # whose completion receipt is faster than SWDGE
                    store_eng = nc.sync if o >= 4 else nc.gpsimd
                    store_eng.dma_start(
                        out=outt[
                            o * 128 : (o + 1) * 128,
                            b * 512 + c * 256 : b * 512 + (c + 1) * 256,
                        ],
                        in_=o_sb[:, cc],
                    )


_NC_CACHE = None


def _build_nc():
    global _NC_CACHE
    if _NC_CACHE is not None:
        return _NC_CACHE
    nc = bacc.Bacc(
        "TRN2", target_bir_lowering=False, debug=False, num_devices=NCORES
    )
    xh = nc.dram_tensor("xh", [H, 2 * BL], BF16, kind="ExternalInput").ap()
    htf = nc.dram_tensor("htf", [H, BL], F32, kind="ExternalInput").ap()
    wp = nc.dram_tensor("wp", [OT, H, 6 * 128], BF16, kind="ExternalInput").ap()
    bias = nc.dram_tensor("bias", [128, OT * 4], F32, kind="ExternalInput").ap()
    outt = nc.dram_tensor("outt", [H, BL], F32, kind="ExternalOutput").ap()

    with tile.TileContext(nc) as tc:
        _gru_tile_kernel(tc, outt, xh, htf, wp, bias)
    nc.compile()
    _NC_CACHE = nc
    return nc


def _pack_inputs(x, h, W_ih_w, W_ih_b, U_r_w, U_z_w, U_n_w, U_n_b):
    x = np.asarray(x, dtype=np.float32)
    h = np.asarray(h, dtype=np.float32)
    xT = np.ascontiguousarray(x.T)                      # [H, B]
    hT = np.ascontiguousarray(h.T)
    xTb = xT.astype(BF16_NP)
    hTb = hT.astype(BF16_NP)

    W_all = np.concatenate(
        [np.asarray(W_ih_w, np.float32)] +
        [np.asarray(u, np.float32) for u in (U_r_w, U_z_w, U_n_w)],
        axis=0,
    )                                                   # [6H, H] rows: Wr Wz Wn Ur Uz Un
    WT = np.ascontiguousarray(W_all.T)                  # [H, 6H], col blocks same order
    # wp[o, k, g*128 + m] = WT[k, g*H + o*128 + m]
    wp = np.ascontiguousarray(
        WT.reshape(H, 6, OT, 128).transpose(2, 0, 1, 3).reshape(OT, H, 6 * 128)
    ).astype(BF16_NP)

    b_all = np.concatenate(
        [np.asarray(W_ih_b, np.float32), np.asarray(U_n_b, np.float32)]
    )                                                   # [4H]: b_r b_z b_n1 b_n2
    # bias[m, o*4 + g] = b_all[g*H + o*128 + m]
    bias = np.ascontiguousarray(
        b_all.reshape(4, OT, 128).transpose(2, 1, 0).reshape(128, OT * 4)
    ).astype(np.float32)

    in_maps = []
    for c in range(NCORES):
        sl = slice(c * BL, (c + 1) * BL)
        xc, hc = xTb[:, sl], hTb[:, sl]
        # per-k rows packed as [x_b0 | h_b0 | x_b1 | h_b1]
        xhc = np.concatenate(
            [xc[:, 0:512], hc[:, 0:512], xc[:, 512:1024], hc[:, 512:1024]],
            axis=1,
        )
        in_maps.append({
            "xh": np.ascontiguousarray(xhc),
            "htf": np.ascontiguousarray(hT[:, sl]),
            "wp": wp,
            "bias": bias,
        })
    return in_maps


def kernel(x, h, W_ih_w, W_ih_b, U_r_w, U_z_w, U_n_w, U_n_b):
    global LAST_RESULT
    nc = _build_nc()
    in_maps = _pack_inputs(x, h, W_ih_w, W_ih_b, U_r_w, U_z_w, U_n_w, U_n_b)
    trace = bool(os.environ.get("GRU_TRACE"))
    res = run_bass_kernel_spmd(nc, in_maps, list(range(NCORES)), trace=trace)
    LAST_RESULT = res
    out = np.empty((B, H), dtype=np.float32)
    for c in range(NCORES):
        out[c * BL : (c + 1) * BL, :] = res.results[c]["outt"].T
    return out
